# revision 4
# baseline (speedup 1.0000x reference)
"""BitLinear (int8-activation x ternary-weight) matmul on 8 TRN2 NeuronCores.

Full inputs: x [4, 4096, 2048] f32, weight [2048, 2048] f32.
Output: [4, 4096, 2048] fp16.

Strategy: data-parallel over the 16384 rows (2048 rows/core). The weight is
replicated; each core quantizes it on-device (mean|W| -> sw -> ternary bf16)
and quantizes its activation rows per-row to int8 values held in bf16
(exact: |qx| <= 127, products accumulate in fp32 PSUM -> exact integers).
Host only reshapes/shards and transposes W (layout prep, no math).
"""

import numpy as np

import concourse.bass as bass
import concourse.mybir as mybir
import concourse.tile as tile
from concourse import bacc
from concourse.bass import ts
from concourse.bass_utils import run_bass_kernel_spmd
from concourse.masks import make_identity

N_CORES = 8
ROWS_TOTAL = 4 * 4096
K = 2048
N = 2048
MAGIC = 12582912.0  # 1.5*2^23: fp32 round-to-nearest-even trick (both signs)

f32 = mybir.dt.float32
bf16 = mybir.dt.bfloat16
f16 = mybir.dt.float16
Alu = mybir.AluOpType
Act = mybir.ActivationFunctionType
AxX = mybir.AxisListType.X


def build(rows_per_core=ROWS_TOTAL // N_CORES):
    nc = bacc.Bacc(
        "TRN2", target_bir_lowering=False, debug=False, num_devices=N_CORES
    )
    x_ext = nc.declare_dram_parameter("x", [rows_per_core, K], f32, isOutput=False)
    wt_ext = nc.declare_dram_parameter("wt", [K, N], f32, isOutput=False)
    out_ext = nc.declare_dram_parameter(
        "out", [rows_per_core, N], f16, isOutput=True
    )

    KT = K // 128
    MT = rows_per_core // 128
    NQ = N // 512

    with tile.TileContext(nc) as tc:
        with (
            tc.tile_pool(name="big", bufs=3) as big,  # [128,K] f32 streaming loads
            tc.tile_pool(name="scaled", bufs=2) as scaled,  # [128,K] f32 ACT-scaled
            tc.tile_pool(name="qtmp", bufs=2) as qtmp,  # rounded f32 / qx bf16
            tc.tile_pool(name="qxt", bufs=2) as qxtp,  # [128,KT,128] bf16 x^T
            tc.tile_pool(name="outp", bufs=3) as outp,  # [128,N] f16 results
            tc.tile_pool(name="singles", bufs=1) as singles,
            tc.tile_pool(name="small", bufs=6) as small,  # [128,1] stats
            tc.tile_pool(name="pacc", bufs=6, space="PSUM") as pacc,
            tc.tile_pool(name="pt", bufs=2, space="PSUM") as pt,
        ):
            ident = singles.tile([128, 128], bf16)
            make_identity(nc, ident)
            ones_col = singles.tile([128, 1], f32)
            nc.vector.memset(ones_col, 1.0)
            ones_row = singles.tile([1, 128], f32)
            nc.vector.memset(ones_row, 1.0)
            qwT = singles.tile([128, KT, N], bf16)
            wsums = singles.tile([128, KT], f32)

            # ---- W pass 1: total |W| -> sw scalars
            for kt in range(KT):
                wt_t = big.tile([128, K], f32, tag="big")
                nc.sync.dma_start(out=wt_t, in_=wt_ext[ts(kt, 128), :])
                nc.vector.tensor_reduce(
                    out=wsums[:, kt : kt + 1],
                    in_=wt_t,
                    axis=AxX,
                    op=Alu.add,
                    apply_absolute_value=True,
                )
            wtot = small.tile([128, 1], f32, tag="small")
            nc.vector.tensor_reduce(out=wtot, in_=wsums, axis=AxX, op=Alu.add)
            ptot = pt.tile([1, 1], f32, tag="pt")
            nc.tensor.matmul(ptot, lhsT=ones_col, rhs=wtot, start=True, stop=True)
            # meanc = max(mean|W|, 1e-5); sw = 1/meanc; q = meanc/127
            s_meanc = small.tile([1, 1], f32, tag="s1")
            nc.vector.tensor_scalar(
                out=s_meanc,
                in0=ptot,
                scalar1=1.0 / (K * N),
                scalar2=1e-5,
                op0=Alu.mult,
                op1=Alu.max,
            )
            s_sw = small.tile([1, 1], f32, tag="s1")
            nc.vector.reciprocal(out=s_sw, in_=s_meanc)
            s_q = small.tile([1, 1], f32, tag="s1")
            nc.vector.tensor_scalar_mul(out=s_q, in0=s_meanc, scalar1=1.0 / 127.0)
            # broadcast scalars to all 128 partitions via PE outer product
            pb = pt.tile([128, 1], f32, tag="pt")
            nc.tensor.matmul(pb, lhsT=ones_row, rhs=s_sw, start=True, stop=True)
            sw_b = singles.tile([128, 1], f32)
            nc.vector.tensor_copy(out=sw_b, in_=pb)
            pb2 = pt.tile([128, 1], f32, tag="pt")
            nc.tensor.matmul(pb2, lhsT=ones_row, rhs=s_q, start=True, stop=True)
            q_b = singles.tile([128, 1], f32)
            nc.vector.tensor_copy(out=q_b, in_=pb2)

            # ---- W pass 2: qwT = clip(round(wT*sw), -1, 1) as bf16
            for kt in range(KT):
                wt_t = big.tile([128, K], f32, tag="big")
                nc.sync.dma_start(out=wt_t, in_=wt_ext[ts(kt, 128), :])
                ws = scaled.tile([128, K], f32, tag="scaled")
                nc.scalar.activation(out=ws, in_=wt_t, func=Act.Copy, scale=sw_b)
                wr = qtmp.tile([128, K], f32, tag="qtmp")
                nc.vector.tensor_scalar(
                    out=wr, in0=ws, scalar1=MAGIC, scalar2=-MAGIC,
                    op0=Alu.add, op1=Alu.add,
                )
                nc.vector.tensor_scalar(
                    out=qwT[:, kt, :], in0=wr, scalar1=-1.0, scalar2=1.0,
                    op0=Alu.max, op1=Alu.min,
                )

            # ---- main loop over row tiles
            for mi in range(MT):
                x_t = big.tile([128, K], f32, tag="big")
                nc.sync.dma_start(out=x_t, in_=x_ext[ts(mi, 128), :])
                amax = small.tile([128, 1], f32, tag="small")
                nc.vector.tensor_reduce(
                    out=amax, in_=x_t, axis=AxX, op=Alu.max,
                    apply_absolute_value=True,
                )
                amc = small.tile([128, 1], f32, tag="small")
                nc.vector.tensor_scalar_max(out=amc, in0=amax, scalar1=1e-5)
                rec = small.tile([128, 1], f32, tag="small")
                nc.vector.reciprocal(out=rec, in_=amc)
                si = small.tile([128, 1], f32, tag="small")
                nc.vector.tensor_scalar_mul(out=si, in0=rec, scalar1=127.0)
                cs = small.tile([128, 1], f32, tag="small")
                nc.vector.tensor_mul(cs, amc, q_b)  # (amax/127)*meanc

                xs = scaled.tile([128, K], f32, tag="scaled")
                nc.scalar.activation(out=xs, in_=x_t, func=Act.Copy, scale=si)
                qx = qtmp.tile([128, K], bf16, tag="qtmp")
                nc.vector.tensor_scalar(
                    out=qx, in0=xs, scalar1=MAGIC, scalar2=-MAGIC,
                    op0=Alu.add, op1=Alu.add,
                )
                qxT = qxtp.tile([128, KT, 128], bf16, tag="qxt")
                for kt in range(KT):
                    ptr = pt.tile([128, 128], bf16, tag="pt")
                    nc.tensor.transpose(ptr, qx[:, ts(kt, 128)], ident)
                    nc.vector.tensor_copy(out=qxT[:, kt, :], in_=ptr)

                accs = [
                    pacc.tile([128, 512], f32, tag="acc", name=f"acc_{mi}_{i}")
                    for i in range(NQ)
                ]
                for nq in range(NQ):
                    for kt in range(KT):
                        nc.tensor.matmul(
                            accs[nq],
                            lhsT=qxT[:, kt, :],
                            rhs=qwT[:, kt, ts(nq, 512)],
                            start=(kt == 0),
                            stop=(kt == KT - 1),
                        )
                o_t = outp.tile([128, N], f16, tag="outp")
                for nq in range(NQ):
                    nc.scalar.activation(
                        out=o_t[:, ts(nq, 512)], in_=accs[nq],
                        func=Act.Copy, scale=cs,
                    )
                nc.sync.dma_start(out=out_ext[ts(mi, 128), :], in_=o_t)

    nc.compile()
    return nc


_NC_CACHE = {}


def _get_nc(rows_per_core):
    if rows_per_core not in _NC_CACHE:
        _NC_CACHE[rows_per_core] = build(rows_per_core)
    return _NC_CACHE[rows_per_core]


def run(x, weight, **spmd_kwargs):
    x = np.ascontiguousarray(np.asarray(x, dtype=np.float32))
    weight = np.asarray(weight, dtype=np.float32)
    b, s, k = x.shape
    rows = b * s
    rpc = rows // N_CORES
    xr = x.reshape(rows, k)
    wt = np.ascontiguousarray(weight.T)
    nc = _get_nc(rpc)
    in_maps = [
        {"x": xr[i * rpc : (i + 1) * rpc], "wt": wt} for i in range(N_CORES)
    ]
    res = run_bass_kernel_spmd(
        nc, in_maps, core_ids=list(range(N_CORES)), **spmd_kwargs
    )
    out = np.concatenate(
        [res.results[i]["out"] for i in range(N_CORES)], axis=0
    )
    return out.reshape(b, s, N), res


def kernel(x, weight):
    out, _ = run(x, weight)
    return out
